# revision 6
# baseline (speedup 1.0000x reference)
"""Multi-head self-attention Trainium2 kernel (B=4, S=2048, D=1024, H=16, dk=64).

Sharding (8 cores): data-parallel over batch (4) x tensor-parallel over head
groups (2).  Core c handles batch c//2 and heads [8*(c%2), 8*(c%2)+8), i.e.
feature columns [512*(c%2), 512*(c%2)+512) of Wq/Wk/Wv (column split) and the
matching rows of Wo (row split).  Each core emits a partial [2048, 1024]
output; the host sums the two partials per batch and adds bo.

Device-side layout (per core), all SBUF data bf16 (psum fp32):
  - x^T (host-pretransposed, host-cast bf16 [1024, 2048]) streams in once.
  - Q^T, K^T computed feature-major [512, 2048] (f on partitions); the
    scores matmul K_h Q_h^T then needs no on-device transposes.  1/sqrt(dk)
    is folded into Wq/bq on the host.  QK biases are added by the DVE
    during psum->SBUF evacuation (per-partition scalar), not on the PE.
  - The head pair (2t, 2t+1) lives on partitions 0-63 / 64-127 of QT/KT
    tile t, so the two scores matmuls auto-derive tile_position (0,0) and
    (64,0) and run concurrently on the PE array halves.
  - V natural [2048, 512] with a ones-column per head: the attention
    matmul V65^T @ E yields EV^T (64 rows) + softmax denominator (row 64).
  - exp on ACT engine (the kernel bottleneck: ~1 elem/cycle/partition),
    one [128, 1024] instruction per (head-pair, kt), double-buffered psum
    so ACT stays saturated.
  - Softmax normalize: reciprocal_approx_fast (single-pass custom DVE op)
    + gpsimd partition broadcast + DVE multiply.
  - Work is emitted head-pair-major so attention on pair t overlaps the
    projections for pair t+1 and the output projection of the last pair.
"""

import numpy as np
import ml_dtypes

import concourse.bass as bass
import concourse.mybir as mybir
import concourse.tile as tile
from concourse import bacc
from concourse.bass_utils import run_bass_kernel_spmd

F32 = mybir.dt.float32
BF16 = mybir.dt.bfloat16

P = 128
D = 1024          # model dim
S = 2048          # sequence length
FH = 512          # local feature width (8 heads x 64)
H_LOC = 8         # heads per core
DK = 64           # head dim
N_DT = D // P     # 8 d-tiles
N_FT = FH // P    # 4 local feature tiles (== head pairs)
N_ST = S // P     # 16 sequence tiles
N_SC = S // 512   # 4 sequence chunks of 512
QC = 512          # query chunk

BF = ml_dtypes.bfloat16


def _emit(nc, tc, xT, wq, bqp, wk, bkp, wv, bv, wo, ones_d, out, dumps=None):
    Exp = mybir.ActivationFunctionType.Exp
    Mult = mybir.AluOpType.mult

    with tc.tile_pool(name="consts", bufs=1) as consts, \
         tc.tile_pool(name="persist", bufs=1) as persist:
        ones = consts.tile([1, 512], BF16)
        nc.sync.dma_start(out=ones, in_=ones_d[:, :])
        bq_sb = consts.tile([P, N_FT], F32)
        nc.sync.dma_start(out=bq_sb, in_=bqp[:, :])
        bk_sb = consts.tile([P, N_FT], F32)
        nc.sync.dma_start(out=bk_sb, in_=bkp[:, :])
        bv_sb = consts.tile([1, FH], BF16)
        nc.sync.dma_start(out=bv_sb, in_=bv[:, :])

        QT = persist.tile([P, N_FT, S], BF16, tag="QT")
        KT = persist.tile([P, N_FT, S], BF16, tag="KT")
        V65 = persist.tile([P, N_ST, H_LOC, DK + 1], BF16, tag="V65")
        AO = persist.tile([P, N_FT, S], BF16, tag="AO")
        wo_sb = persist.tile([P, N_FT, D], BF16, tag="wo")
        nc.vector.memset(V65[:, :, :, DK:DK + 1], 1.0)

        with tc.tile_pool(name="xt_pool", bufs=N_DT) as xt_pool, \
             tc.tile_pool(name="w_pool", bufs=3) as w_pool, \
             tc.tile_pool(name="ps_small", bufs=2, space="PSUM") as ps_small, \
             tc.tile_pool(name="psSC", bufs=2, space="PSUM") as psSC, \
             tc.tile_pool(name="psEV", bufs=2, space="PSUM") as psEV, \
             tc.tile_pool(name="e_pool", bufs=4) as e_pool, \
             tc.tile_pool(name="r_pool", bufs=4) as r_pool, \
             tc.tile_pool(name="o_pool", bufs=4) as o_pool:

            wk_sb = w_pool.tile([P, N_DT, FH], BF16, tag="w", name="wks")
            nc.sync.dma_start(
                out=wk_sb, in_=wk[:, :].rearrange("(dt p) f -> p dt f", p=P))
            xts = []
            for dt in range(N_DT):
                xt = xt_pool.tile([P, S], BF16, tag="xt", name=f"xt{dt}")
                nc.sync.dma_start(out=xt, in_=xT[dt * P:(dt + 1) * P, :])
                xts.append(xt)
            wv_sb = w_pool.tile([P, N_DT, FH], BF16, tag="w", name="wvs")
            nc.sync.dma_start(
                out=wv_sb, in_=wv[:, :].rearrange("(dt p) f -> p dt f", p=P))
            wq_sb = w_pool.tile([P, N_DT, FH], BF16, tag="w", name="wqs")
            nc.sync.dma_start(
                out=wq_sb, in_=wq[:, :].rearrange("(dt p) f -> p dt f", p=P))
            nc.sync.dma_start(
                out=wo_sb, in_=wo[:, :].rearrange("(ft p) e -> p ft e", p=P))

            def kq_block(w_sb, b_sb, dest, t, sc):
                # dest[f, s] = sum_d W[d, f] x^T[d, s]; bias added on DVE.
                ps = ps_small.tile([P, QC], F32, tag="pss", name="pskq")
                for dt in range(N_DT):
                    nc.tensor.matmul(
                        ps,
                        w_sb[:, dt, t * P:(t + 1) * P],
                        xts[dt][:, sc * QC:(sc + 1) * QC],
                        start=(dt == 0), stop=(dt == N_DT - 1))
                nc.vector.tensor_scalar_add(
                    out=dest[:, t, sc * QC:(sc + 1) * QC], in0=ps,
                    scalar1=b_sb[:, t:t + 1])

            def v_block(st):
                ps = ps_small.tile([P, FH], F32, tag="pss", name="psv")
                for dt in range(N_DT):
                    nc.tensor.matmul(
                        ps,
                        xts[dt][:, st * P:(st + 1) * P],
                        wv_sb[:, dt, :],
                        start=(dt == 0), stop=False)
                nc.tensor.matmul(
                    ps, ones[:, 0:P], bv_sb, start=False, stop=True)
                nc.vector.tensor_copy(
                    out=V65[:, st, :, 0:DK],
                    in_=ps[:, :].rearrange("p (h d) -> p h d", h=H_LOC))

            def attn_block(t, qc):
                evs = [psEV.tile([DK + 1, QC], F32, tag="ev", name=f"ev{h2}")
                       for h2 in range(2)]
                for kt in range(N_ST):
                    ps = psSC.tile([P, 2 * QC], F32, tag="sc", name="scps")
                    for h2 in range(2):
                        lo = h2 * DK
                        nc.tensor.matmul(
                            ps[:, h2 * QC:(h2 + 1) * QC],
                            KT[lo:lo + DK, t, kt * P:(kt + 1) * P],
                            QT[lo:lo + DK, t, qc * QC:(qc + 1) * QC],
                            start=True, stop=True, skip_group_check=True)
                    e = e_pool.tile([P, 2 * QC], BF16, tag="e", name="esb")
                    nc.scalar.activation(out=e, in_=ps, func=Exp)
                    for h2 in range(2):
                        nc.tensor.matmul(
                            evs[h2],
                            V65[:, kt, 2 * t + h2, :],
                            e[:, h2 * QC:(h2 + 1) * QC],
                            start=(kt == 0), stop=(kt == N_ST - 1),
                            skip_group_check=True)
                for h2 in range(2):
                    r1 = r_pool.tile([1, QC], F32, tag="r1", name="r1")
                    nc.vector.reciprocal(out=r1, in_=evs[h2][DK:DK + 1, :])
                    rb = r_pool.tile([DK, QC], F32, tag="rb", name="rb")
                    nc.gpsimd.partition_broadcast(rb, r1)
                    nc.vector.tensor_mul(
                        out=AO[h2 * DK:(h2 + 1) * DK, t,
                               qc * QC:(qc + 1) * QC],
                        in0=evs[h2][0:DK, :], in1=rb)

            def out_block(qc):
                # out[s, e] = sum_f AO[f, s] wo[f, e] for the 4 st tiles of qc
                for sti in range(4):
                    st = qc * 4 + sti
                    for ec in range(D // QC):
                        ps = ps_small.tile([P, QC], F32, tag="pss", name="pso")
                        for ft in range(N_FT):
                            nc.tensor.matmul(
                                ps,
                                AO[:, ft, st * P:(st + 1) * P],
                                wo_sb[:, ft, ec * QC:(ec + 1) * QC],
                                start=(ft == 0), stop=(ft == N_FT - 1))
                        ob = o_pool.tile([P, QC], F32, tag="ob", name="ob")
                        nc.vector.tensor_copy(out=ob, in_=ps)
                        nc.sync.dma_start(
                            out=out[st * P:(st + 1) * P,
                                    ec * QC:(ec + 1) * QC],
                            in_=ob)

            # ---- emission order: K(t)/Q(t,qc) just-in-time, V once ----
            for t in range(N_FT):
                for sc in range(N_SC):
                    kq_block(wk_sb, bk_sb, KT, t, sc)
                if t == 0:
                    for st in range(N_ST):
                        v_block(st)
                for qc in range(N_SC):
                    kq_block(wq_sb, bq_sb, QT, t, qc)
                    attn_block(t, qc)
                    if t == N_FT - 1:
                        out_block(qc)

            if dumps is not None:
                nc.sync.dma_start(
                    out=dumps["qt_d"],
                    in_=QT[:, :, :].rearrange("p a b -> p (a b)"))
                nc.sync.dma_start(
                    out=dumps["kt_d"],
                    in_=KT[:, :, :].rearrange("p a b -> p (a b)"))
                nc.sync.dma_start(
                    out=dumps["v_d"],
                    in_=V65[:, :, :, :].rearrange("p a b c -> p (a b c)"))
                nc.sync.dma_start(
                    out=dumps["ao_d"],
                    in_=AO[:, :, :].rearrange("p a b -> p (a b)"))


def build_nc(debug=False, repeat=1, dump=False):
    nc = bacc.Bacc("TRN2", debug=debug)
    xT = nc.declare_dram_parameter("xT", [D, S], BF16, isOutput=False)
    wq = nc.declare_dram_parameter("wq", [D, FH], BF16, isOutput=False)
    bqp = nc.declare_dram_parameter("bqp", [P, N_FT], F32, isOutput=False)
    wk = nc.declare_dram_parameter("wk", [D, FH], BF16, isOutput=False)
    bkp = nc.declare_dram_parameter("bkp", [P, N_FT], F32, isOutput=False)
    wv = nc.declare_dram_parameter("wv", [D, FH], BF16, isOutput=False)
    bv = nc.declare_dram_parameter("bv", [1, FH], BF16, isOutput=False)
    wo = nc.declare_dram_parameter("wo", [FH, D], BF16, isOutput=False)
    ones_d = nc.declare_dram_parameter("ones_d", [1, 512], BF16, isOutput=False)
    out = nc.declare_dram_parameter("out", [S, D], F32, isOutput=True)
    dumps = None
    if dump:
        dumps = {
            "qt_d": nc.declare_dram_parameter(
                "qt_d", [P, N_FT * S], BF16, isOutput=True)[:, :],
            "kt_d": nc.declare_dram_parameter(
                "kt_d", [P, N_FT * S], BF16, isOutput=True)[:, :],
            "v_d": nc.declare_dram_parameter(
                "v_d", [P, N_ST * H_LOC * (DK + 1)], BF16, isOutput=True)[:, :],
            "ao_d": nc.declare_dram_parameter(
                "ao_d", [P, N_FT * S], BF16, isOutput=True)[:, :],
        }
    with tile.TileContext(nc) as tc:
        for _rep in range(repeat):
            _emit(nc, tc, xT[:, :], wq[:, :], bqp[:, :], wk[:, :], bkp[:, :],
                  wv[:, :], bv[:, :], wo[:, :], ones_d[:, :], out[:, :],
                  dumps=dumps)
    nc.compile()
    return nc


def make_in_maps(x, Wq, bq, Wk, bk, Wv, bv, Wo):
    in_maps = []
    for c in range(8):
        b, hg = divmod(c, 2)
        F = slice(FH * hg, FH * (hg + 1))
        in_maps.append({
            "xT": np.ascontiguousarray(x[b].T).astype(BF),
            "wq": (np.ascontiguousarray(Wq[:, F]) * 0.125).astype(BF),
            "bqp": np.ascontiguousarray(
                (bq[F] * 0.125).reshape(N_FT, P).T),
            "wk": np.ascontiguousarray(Wk[:, F]).astype(BF),
            "bkp": np.ascontiguousarray(bk[F].reshape(N_FT, P).T),
            "wv": np.ascontiguousarray(Wv[:, F]).astype(BF),
            "bv": bv[F].reshape(1, FH).astype(BF),
            "wo": np.ascontiguousarray(Wo[F, :]).astype(BF),
            "ones_d": np.ones((1, 512), BF),
        })
    return in_maps


_NC_CACHE = None


def _get_nc():
    global _NC_CACHE
    if _NC_CACHE is None:
        _NC_CACHE = build_nc()
    return _NC_CACHE


def kernel(x, Wq, bq, Wk, bk, Wv, bv, Wo, bo, _trace=False):
    x = np.asarray(x, np.float32)
    args = [np.asarray(a, np.float32) for a in (Wq, bq, Wk, bk, Wv, bv, Wo)]
    bo = np.asarray(bo, np.float32)
    nc = _get_nc()
    in_maps = make_in_maps(x, *args)
    res = run_bass_kernel_spmd(nc, in_maps, list(range(8)), trace=_trace)
    out = np.empty((4, S, D), np.float32)
    for b in range(4):
        out[b] = res.results[2 * b]["out"] + res.results[2 * b + 1]["out"] + bo
    if _trace:
        return out, res
    return out


# revision 21
# speedup vs baseline: 1.4459x; 1.4459x over previous
"""Multi-head self-attention Trainium2 kernel (B=4, S=2048, D=1024, H=16, dk=64).

Sharding (8 cores): data-parallel over batch (4) x tensor-parallel over head
groups (2).  Core c handles batch c//2 and heads [8*(c%2), 8*(c%2)+8), i.e.
feature columns [512*(c%2), 512*(c%2)+512) of Wq/Wk/Wv (column split) and the
matching rows of Wo (row split).  Each core emits a partial [2048, 1024]
output; the host sums the two partials per batch and adds bo.

Device-side layout (per core), all SBUF data bf16 (psum fp32):
  - x^T (host-pretransposed, host-cast bf16 [1024, 2048]) streams in once.
  - Q^T, K^T computed feature-major [512, 2048] (f on partitions); the
    scores matmul K_h Q_h^T then needs no on-device transposes.  1/sqrt(dk)
    is folded into Wq/bq on the host.  QK biases are added by the DVE
    during psum->SBUF evacuation (per-partition scalar), not on the PE.
  - The head pair (2t, 2t+1) lives on partitions 0-63 / 64-127 of QT/KT
    tile t, so the two scores matmuls auto-derive tile_position (0,0) and
    (64,0) and run concurrently on the PE array halves.
  - V natural [2048, 512] with a ones-column per head: the attention
    matmul V65^T @ E yields EV^T (64 rows) + softmax denominator (row 64).
  - exp on ACT engine (the kernel bottleneck: ~1 elem/cycle/partition),
    one [128, 1024] instruction per (head-pair, kt), double-buffered psum
    so ACT stays saturated.
  - Softmax normalize: reciprocal_approx_fast (single-pass custom DVE op)
    + gpsimd partition broadcast + DVE multiply.
  - Work is emitted head-pair-major so attention on pair t overlaps the
    projections for pair t+1 and the output projection of the last pair.
"""

import numpy as np
import ml_dtypes

import concourse.bass as bass
import concourse.mybir as mybir
import concourse.tile as tile
from concourse import bacc
from concourse.bass_utils import run_bass_kernel_spmd

F32 = mybir.dt.float32
BF16 = mybir.dt.bfloat16

P = 128
D = 1024          # model dim
S = 2048          # sequence length
FH = 512          # local feature width (8 heads x 64)
H_LOC = 8         # heads per core
DK = 64           # head dim
N_DT = D // P     # 8 d-tiles
N_FT = FH // P    # 4 local feature tiles (== head pairs)
N_ST = S // P     # 16 sequence tiles
N_SC = S // 512   # 4 sequence chunks of 512
QC = 512          # query chunk

BF = ml_dtypes.bfloat16


def _emit(nc, tc, xT, wq, bqp, wk, bkp, wv, bv, wo, ones_d, out, dumps=None,
          stages=4):
    Exp = mybir.ActivationFunctionType.Exp
    Mult = mybir.AluOpType.mult

    with tc.tile_pool(name="consts", bufs=1) as consts, \
         tc.tile_pool(name="persist", bufs=1) as persist:
        ones = consts.tile([1, 512], BF16)
        nc.sync.dma_start(out=ones, in_=ones_d[:, :])
        bq_sb = consts.tile([P, N_FT], F32)
        nc.sync.dma_start(out=bq_sb, in_=bqp[:, :])
        bk_sb = consts.tile([P, N_FT], F32)
        nc.sync.dma_start(out=bk_sb, in_=bkp[:, :])
        bv_sb = consts.tile([1, FH], BF16)
        nc.sync.dma_start(out=bv_sb, in_=bv[:, :])

        QT = persist.tile([P, N_FT, S], BF16, tag="QT")
        KT = persist.tile([P, N_FT, S], BF16, tag="KT")
        V65 = persist.tile([P, N_ST, H_LOC, DK + 1], BF16, tag="V65")
        AO = persist.tile([P, N_FT, S], BF16, tag="AO")
        wo_sb = persist.tile([P, N_FT, D], BF16, tag="wo")
        nc.vector.memset(V65[:, :, :, DK:DK + 1], 1.0)

        with tc.tile_pool(name="xt_pool", bufs=N_DT) as xt_pool, \
             tc.tile_pool(name="w_pool", bufs=3) as w_pool, \
             tc.tile_pool(name="ps_small", bufs=2, space="PSUM") as ps_small, \
             tc.tile_pool(name="psSC", bufs=2, space="PSUM") as psSC, \
             tc.tile_pool(name="psEV", bufs=2, space="PSUM") as psEV, \
             tc.tile_pool(name="e_pool", bufs=4) as e_pool, \
             tc.tile_pool(name="r_pool", bufs=4) as r_pool, \
             tc.tile_pool(name="o_pool", bufs=4) as o_pool:

            wk_sb = w_pool.tile([P, N_DT, FH], BF16, tag="w", name="wks")
            nc.sync.dma_start(
                out=wk_sb, in_=wk[:, :].rearrange("(dt p) f -> p dt f", p=P))
            xts = []
            for dt in range(N_DT):
                xt = xt_pool.tile([P, S], BF16, tag="xt", name=f"xt{dt}")
                nc.sync.dma_start(out=xt, in_=xT[dt * P:(dt + 1) * P, :])
                xts.append(xt)
            wv_sb = w_pool.tile([P, N_DT, FH], BF16, tag="w", name="wvs")
            nc.sync.dma_start(
                out=wv_sb, in_=wv[:, :].rearrange("(dt p) f -> p dt f", p=P))
            wq_sb = w_pool.tile([P, N_DT, FH], BF16, tag="w", name="wqs")
            nc.sync.dma_start(
                out=wq_sb, in_=wq[:, :].rearrange("(dt p) f -> p dt f", p=P))
            nc.sync.dma_start(
                out=wo_sb, in_=wo[:, :].rearrange("(ft p) e -> p ft e", p=P))

            def kq_block(w_sb, b_sb, dest, t, sc):
                # dest[f, s] = sum_d W[d, f] x^T[d, s]; bias added on DVE.
                ps = ps_small.tile([P, QC], F32, tag="pss", name="pskq")
                for dt in range(N_DT):
                    nc.tensor.matmul(
                        ps,
                        w_sb[:, dt, t * P:(t + 1) * P],
                        xts[dt][:, sc * QC:(sc + 1) * QC],
                        start=(dt == 0), stop=(dt == N_DT - 1))
                nc.vector.tensor_scalar_add(
                    out=dest[:, t, sc * QC:(sc + 1) * QC], in0=ps,
                    scalar1=b_sb[:, t:t + 1])

            def v_block(st):
                ps = ps_small.tile([P, FH], F32, tag="pss", name="psv")
                for dt in range(N_DT):
                    nc.tensor.matmul(
                        ps,
                        xts[dt][:, st * P:(st + 1) * P],
                        wv_sb[:, dt, :],
                        start=(dt == 0), stop=False)
                nc.tensor.matmul(
                    ps, ones[:, 0:P], bv_sb, start=False, stop=True)
                nc.vector.tensor_copy(
                    out=V65[:, st, :, 0:DK],
                    in_=ps[:, :].rearrange("p (h d) -> p h d", h=H_LOC))

            def attn_block(t, qc, fillers=()):
                # Emit scores with a 2-kt lookahead: the EV matmul for kt
                # waits on exp(kt), and the PE queue is FIFO — emitting
                # scores(kt+2) before ev(kt) would leave them stuck behind
                # it; emitting them ahead keeps the PE producing while ACT
                # works, so ACT (the bottleneck) stays back-to-back.
                evs = [psEV.tile([DK + 1, QC], F32, tag="ev", name=f"ev{h2}")
                       for h2 in range(2)]

                def scores(kt):
                    ps = psSC.tile([P, 2 * QC], F32, tag="sc", name="scps")
                    for h2 in range(2):
                        lo = h2 * DK
                        nc.tensor.matmul(
                            ps[:, h2 * QC:(h2 + 1) * QC],
                            KT[lo:lo + DK, t, kt * P:(kt + 1) * P],
                            QT[lo:lo + DK, t, qc * QC:(qc + 1) * QC],
                            start=True, stop=True, skip_group_check=True)
                    return ps

                fillers = list(fillers)
                pss = {0: scores(0), 1: scores(1)}
                for kt in range(N_ST):
                    if stages < 2:
                        if kt + 2 < N_ST:
                            pss[kt + 2] = scores(kt + 2)
                        continue
                    e = e_pool.tile([P, 2 * QC], BF16, tag="e", name="esb")
                    nc.scalar.activation(out=e, in_=pss.pop(kt), func=Exp)
                    if stages >= 3:
                        for h2 in range(2):
                            nc.tensor.matmul(
                                evs[h2],
                                V65[:, kt, 2 * t + h2, :],
                                e[:, h2 * QC:(h2 + 1) * QC],
                                start=(kt == 0), stop=(kt == N_ST - 1),
                                skip_group_check=True)
                    if kt + 2 < N_ST:
                        pss[kt + 2] = scores(kt + 2)
                    if fillers and kt % 2 == 1:
                        fillers.pop(0)()
                if stages < 3:
                    return
                for h2 in range(2):
                    r1 = r_pool.tile([1, QC], F32, tag="r1", name="r1")
                    nc.vector.reciprocal(out=r1, in_=evs[h2][DK:DK + 1, :])
                    rb = r_pool.tile([DK, QC], F32, tag="rb", name="rb")
                    nc.gpsimd.partition_broadcast(rb, r1)
                    nc.vector.tensor_mul(
                        out=AO[h2 * DK:(h2 + 1) * DK, t,
                               qc * QC:(qc + 1) * QC],
                        in0=evs[h2][0:DK, :], in1=rb)

            def out_tile(st, ec):
                # out[s, e] = sum_f AO[f, s] wo[f, e]
                ps = ps_small.tile([P, QC], F32, tag="pss", name="pso")
                for ft in range(N_FT):
                    nc.tensor.matmul(
                        ps,
                        AO[:, ft, st * P:(st + 1) * P],
                        wo_sb[:, ft, ec * QC:(ec + 1) * QC],
                        start=(ft == 0), stop=(ft == N_FT - 1))
                ob = o_pool.tile([P, QC], F32, tag="ob", name="ob")
                nc.vector.tensor_copy(out=ob, in_=ps)
                nc.sync.dma_start(
                    out=out[st * P:(st + 1) * P, ec * QC:(ec + 1) * QC],
                    in_=ob)

            def out_thunks(qc):
                return [
                    (lambda st=qc * 4 + sti, ec=ec: out_tile(st, ec))
                    for sti in range(4) for ec in range(D // QC)]

            def out_block(qc):
                for th in out_thunks(qc):
                    th()

            # ---- emission order: K(t)/Q(t,qc) just-in-time, V once ----
            for t in range(N_FT):
                for sc in range(N_SC):
                    kq_block(wk_sb, bk_sb, KT, t, sc)
                if t == 0:
                    for st in range(N_ST):
                        v_block(st)
                for qc in range(N_SC):
                    kq_block(wq_sb, bq_sb, QT, t, qc)
                    if stages >= 1:
                        # C tiles for qc-1 (t==3) are sprinkled into this
                        # attention block's kt loop: they depend only on the
                        # long-finished normalize of qc-1, so they fill PE
                        # gaps without stalling the scores->exp stream.
                        fill = (out_thunks(qc - 1)
                                if stages >= 4 and t == N_FT - 1 and qc > 0
                                else ())
                        attn_block(t, qc, fillers=fill)
                    if t == N_FT - 1 and stages >= 4:
                        if qc == N_SC - 1:
                            out_block(qc)
                    elif t == N_FT - 1 and stages < 4:
                        # keep outputs written so the NEFF is valid
                        ob = o_pool.tile([P, QC], F32, tag="ob", name="ob")
                        nc.vector.tensor_copy(
                            out=ob,
                            in_=QT[:, 0, qc * QC:(qc + 1) * QC])
                        nc.sync.dma_start(
                            out=out[qc * P:(qc + 1) * P, 0:QC], in_=ob)

            if dumps is not None:
                nc.sync.dma_start(
                    out=dumps["qt_d"],
                    in_=QT[:, :, :].rearrange("p a b -> p (a b)"))
                nc.sync.dma_start(
                    out=dumps["kt_d"],
                    in_=KT[:, :, :].rearrange("p a b -> p (a b)"))
                nc.sync.dma_start(
                    out=dumps["v_d"],
                    in_=V65[:, :, :, :].rearrange("p a b c -> p (a b c)"))
                nc.sync.dma_start(
                    out=dumps["ao_d"],
                    in_=AO[:, :, :].rearrange("p a b -> p (a b)"))


def build_nc(debug=False, repeat=1, dump=False, stages=4):
    nc = bacc.Bacc("TRN2", debug=debug)
    xT = nc.declare_dram_parameter("xT", [D, S], BF16, isOutput=False)
    wq = nc.declare_dram_parameter("wq", [D, FH], BF16, isOutput=False)
    bqp = nc.declare_dram_parameter("bqp", [P, N_FT], F32, isOutput=False)
    wk = nc.declare_dram_parameter("wk", [D, FH], BF16, isOutput=False)
    bkp = nc.declare_dram_parameter("bkp", [P, N_FT], F32, isOutput=False)
    wv = nc.declare_dram_parameter("wv", [D, FH], BF16, isOutput=False)
    bv = nc.declare_dram_parameter("bv", [1, FH], BF16, isOutput=False)
    wo = nc.declare_dram_parameter("wo", [FH, D], BF16, isOutput=False)
    ones_d = nc.declare_dram_parameter("ones_d", [1, 512], BF16, isOutput=False)
    out = nc.declare_dram_parameter("out", [S, D], F32, isOutput=True)
    dumps = None
    if dump:
        dumps = {
            "qt_d": nc.declare_dram_parameter(
                "qt_d", [P, N_FT * S], BF16, isOutput=True)[:, :],
            "kt_d": nc.declare_dram_parameter(
                "kt_d", [P, N_FT * S], BF16, isOutput=True)[:, :],
            "v_d": nc.declare_dram_parameter(
                "v_d", [P, N_ST * H_LOC * (DK + 1)], BF16, isOutput=True)[:, :],
            "ao_d": nc.declare_dram_parameter(
                "ao_d", [P, N_FT * S], BF16, isOutput=True)[:, :],
        }
    with tile.TileContext(nc) as tc:
        for _rep in range(repeat):
            _emit(nc, tc, xT[:, :], wq[:, :], bqp[:, :], wk[:, :], bkp[:, :],
                  wv[:, :], bv[:, :], wo[:, :], ones_d[:, :], out[:, :],
                  dumps=dumps, stages=stages)
    nc.compile()
    return nc


def make_in_maps(x, Wq, bq, Wk, bk, Wv, bv, Wo):
    in_maps = []
    for c in range(8):
        b, hg = divmod(c, 2)
        F = slice(FH * hg, FH * (hg + 1))
        in_maps.append({
            "xT": np.ascontiguousarray(x[b].T).astype(BF),
            "wq": (np.ascontiguousarray(Wq[:, F]) * 0.125).astype(BF),
            "bqp": np.ascontiguousarray(
                (bq[F] * 0.125).reshape(N_FT, P).T),
            "wk": np.ascontiguousarray(Wk[:, F]).astype(BF),
            "bkp": np.ascontiguousarray(bk[F].reshape(N_FT, P).T),
            "wv": np.ascontiguousarray(Wv[:, F]).astype(BF),
            "bv": bv[F].reshape(1, FH).astype(BF),
            "wo": np.ascontiguousarray(Wo[F, :]).astype(BF),
            "ones_d": np.ones((1, 512), BF),
        })
    return in_maps


_NC_CACHE = None


def _get_nc():
    global _NC_CACHE
    if _NC_CACHE is None:
        _NC_CACHE = build_nc()
    return _NC_CACHE


def kernel(x, Wq, bq, Wk, bk, Wv, bv, Wo, bo, _trace=False):
    x = np.asarray(x, np.float32)
    args = [np.asarray(a, np.float32) for a in (Wq, bq, Wk, bk, Wv, bv, Wo)]
    bo = np.asarray(bo, np.float32)
    nc = _get_nc()
    in_maps = make_in_maps(x, *args)
    res = run_bass_kernel_spmd(nc, in_maps, list(range(8)), trace=_trace)
    out = np.empty((4, S, D), np.float32)
    for b in range(4):
        out[b] = res.results[2 * b]["out"] + res.results[2 * b + 1]["out"] + bo
    if _trace:
        return out, res
    return out
